# revision 1
# baseline (speedup 1.0000x reference)
"""Trainium2 Bass kernel for a transformer attention block (BasicBlock).

Reference computation (B=2, L=2048, D=1024, H=16, C=64):
    qkv = x @ w_qkv.T + b_qkv ; q,k,v = split(qkv)
    attn = softmax((q @ k.T) / sqrt(D)) ; heads = attn @ v
    out  = heads @ w_o.T + b_o + x

Sharding: 8 cores = 2 batches x 4 head-groups (4 heads each).
Per core (b, g):
    qkvT = w_qkv_g @ x_b.T (+bias for q,k at eviction)    [768, 2048]
    V    = transpose(V^T) via PE                          [2048, 4x65]
    S^T_h = zero-padded K=128 matmuls (K^T_h stationary)  per (h, l-chunk)
    P^T  = exp(S^T * scale)   (no max-subtraction; scores bounded ~±1)
    O'^T = [V_h | 1]^T @ P^T  -> rows 0..63 = O^T, row 64 = denominators
    normalize via reciprocal + partition_broadcast, place in ot via SBUF DMA
    partial = O @ w_o[:, cols_g].T                        [2048, 1024]
Host: sum 4 group partials per batch, add x + b_o + w_o @ b_v.
"""

import sys

if "/opt/trn_rl_repo" not in sys.path:
    sys.path.insert(0, "/opt/trn_rl_repo")

import numpy as np

B, L, D, H = 2, 2048, 1024, 16
C = 64
HPC = 4            # heads per core
G = 256            # dims per head group (HPC * C)
SCALE = float(1.0 / np.sqrt(np.float32(D)))

LC = 512           # l-chunk (moving dim)
NLC = L // LC      # 4
MT = L // 128      # 16 m-tiles
DT = D // 128      # 8 d-tiles
NEC = D // 512     # 2 e-chunks for out projection

_CACHE = {}

ALL_PHASES = ("p1", "vt", "scores", "exp", "av", "p4")


def _build(reps=1, phases=ALL_PHASES):
    import concourse.mybir as mybir
    import concourse.tile as tile
    from concourse import bacc
    from concourse.masks import make_identity
    from contextlib import ExitStack

    f32 = mybir.dt.float32
    f32r = mybir.dt.float32r
    Exp = mybir.ActivationFunctionType.Exp

    nc = bacc.Bacc("TRN2", target_bir_lowering=False, debug=False)

    xT = nc.declare_dram_parameter("xT", [D, L], f32r, isOutput=False)
    # columns: [Q (256) | K (256) | V (256)] of this head group, transposed
    wqkvT = nc.declare_dram_parameter("wqkvT", [D, 3 * G], f32r, isOutput=False)
    bqk = nc.declare_dram_parameter("bqk", [128, 4], f32, isOutput=False)
    woT = nc.declare_dram_parameter("woT", [G, D], f32r, isOutput=False)
    out = nc.declare_dram_parameter("out", [L, D], f32, isOutput=True)

    with tile.TileContext(nc) as tc:
      for _rep in range(reps):
        with (
            tc.tile_pool(name="const", bufs=1) as constp,
            tc.tile_pool(name="qp", bufs=2) as qpp,
            tc.tile_pool(name="kz", bufs=4) as kzp,
            tc.tile_pool(name="vt", bufs=16) as vtp,
            tc.tile_pool(name="wo", bufs=2) as wop,
            tc.tile_pool(name="ot", bufs=2) as otp,
            tc.tile_pool(name="ps_mm", bufs=2, space="PSUM") as psmm,
            tc.tile_pool(name="ps_sc", bufs=2, space="PSUM") as pssc,
            tc.tile_pool(name="ps_o", bufs=2, space="PSUM") as pso,
        ):
            bqk_sb = constp.tile([128, 4], f32)
            nc.sync.dma_start(out=bqk_sb[:], in_=bqk[:])

            wo_sb = []
            for t in range(2):
                w = wop.tile([128, D], f32r, name="wo_sb", tag="wo_sb")
                nc.sync.dma_start(out=w[:], in_=woT[t * 128:(t + 1) * 128, :])
                wo_sb.append(w)

            # qp[p]: Q^T pair tiles (partitions 0-63 head 2p, 64-127 head 2p+1)
            qp = [qpp.tile([128, L], f32r, name="qp", tag="qp") for _ in range(2)]
            # kz[h]: K^T_h zero-padded to 128 partitions at its parity offset
            kz = [kzp.tile([128, L], f32r, name="kz", tag="kz") for _ in range(HPC)]
            # v[mt]: [128, 4*65]; per head block: [V_h (64 cols) | ones]
            vt = [vtp.tile([128, HPC * 65], f32r, name="vt", tag="vt") for _ in range(MT)]
            ot = [otp.tile([128, L], f32r, name="ot", tag="ot") for _ in range(2)]

            with (
                tc.tile_pool(name="xt", bufs=DT) as xtp,
                tc.tile_pool(name="wqkv", bufs=DT) as wqkvp,
            ):
                xt, wq = [], []
                for i in range(DT):
                    x_sb = xtp.tile([128, L], f32r, name="x_sb", tag="x_sb")
                    for c in range(NLC):
                        cs = slice(c * LC, (c + 1) * LC)
                        nc.sync.dma_start(out=x_sb[:, cs], in_=xT[i * 128:(i + 1) * 128, cs])
                    xt.append(x_sb)
                    w = wqkvp.tile([128, 3 * G], f32r, name="wqkv_sb", tag="wqkv_sb")
                    nc.sync.dma_start(out=w[:], in_=wqkvT[i * 128:(i + 1) * 128, :])
                    wq.append(w)

                # zero the unused parity halves of kz (memset cannot write
                # f32r; multiply loaded data by 0 instead)
                for h in range(HPC):
                    zs = slice(64, 128) if h % 2 == 0 else slice(0, 64)
                    nc.gpsimd.tensor_scalar_mul(kz[h][zs, :], xt[0][zs, :], 0.0)

                # ---- P1: qkvT = wqkv^T.T @ xT ----
                # t: 0,1 = Q pairs; 2,3 = K pairs; 4,5 = V^T tiles.
                # K first so attention units for heads 0/1 can start early.
                for t in ([2, 0, 3, 1] if "p1" in phases else []):
                    for lc in range(NLC):
                        ps = psmm.tile([128, LC], f32, name="ps", tag="ps")
                        for d in range(DT):
                            nc.tensor.matmul(
                                ps[:],
                                lhsT=wq[d][:, t * 128:(t + 1) * 128],
                                rhs=xt[d][:, lc * LC:(lc + 1) * LC],
                                start=(d == 0),
                                stop=(d == DT - 1),
                            )
                        ls = slice(lc * LC, (lc + 1) * LC)
                        if t < 2:
                            nc.vector.tensor_scalar_add(
                                qp[t][:, ls], ps[:], bqk_sb[:, t:t + 1]
                            )
                        else:
                            h0 = 2 * (t - 2)
                            nc.vector.tensor_scalar_add(
                                kz[h0][0:64, ls], ps[0:64, :], bqk_sb[0:64, t:t + 1]
                            )
                            nc.vector.tensor_scalar_add(
                                kz[h0 + 1][64:128, ls], ps[64:128, :],
                                bqk_sb[64:128, t:t + 1],
                            )

                # ---- P2: V = xT.T @ wv (direct, N=256) ----
                for mt in (range(MT) if "vt" in phases else []):
                    ps = psmm.tile([128, G], f32, name="ps", tag="ps")
                    for d in range(DT):
                        nc.tensor.matmul(
                            ps[:],
                            lhsT=xt[d][:, mt * 128:(mt + 1) * 128],
                            rhs=wq[d][:, 2 * G:3 * G],
                            start=(d == 0),
                            stop=(d == DT - 1),
                        )
                    v3d = vt[mt][:].rearrange("p (h c) -> p h c", h=HPC)
                    nc.vector.tensor_copy(
                        v3d[:, :, 0:64], ps[:].rearrange("p (h c) -> p h c", h=HPC)
                    )
                    nc.vector.tensor_scalar(
                        v3d[:, :, 64:65], v3d[:, :, 0:1], 0.0, 1.0,
                        mybir.AluOpType.mult, mybir.AluOpType.add,
                    )

            _p34 = ExitStack()
            ptp = _p34.enter_context(tc.tile_pool(name="pt", bufs=12))
            rcpp = _p34.enter_context(tc.tile_pool(name="rcp", bufs=3))
            nrmp = _p34.enter_context(tc.tile_pool(name="nrm", bufs=3))
            stgp = _p34.enter_context(tc.tile_pool(name="stg", bufs=4))

            # ---- P3: attention per (head, l-chunk) ----
            pts = None
            for h in (range(HPC) if "scores" in phases else []):
                po_off = (h % 2) * 64
                for lc in range(NLC):
                    ls = slice(lc * LC, (lc + 1) * LC)
                    pts = []
                    for j in range(MT // 2):
                        ps = pssc.tile([128, 2 * LC], f32)
                        for half in range(2):
                            mt = 2 * j + half
                            nc.tensor.matmul(
                                ps[:, half * LC:(half + 1) * LC],
                                lhsT=kz[h][:, mt * 128:(mt + 1) * 128],
                                rhs=qp[h // 2][:, ls],
                                start=True,
                                stop=True,
                            )
                        ptile = ptp.tile([128, 2 * LC], f32r)
                        if "exp" in phases:
                            nc.scalar.activation(ptile[:], ps[:], Exp, scale=SCALE)
                        else:
                            nc.vector.tensor_copy(ptile[:, 0:8], ps[:, 0:8])
                        pts.append(ptile)

                    if "av" not in phases:
                        continue
                    po = pso.tile([65, LC], f32)
                    for j in range(MT // 2):
                        for half in range(2):
                            mt = 2 * j + half
                            nc.tensor.matmul(
                                po[:],
                                lhsT=vt[mt][:, h * 65:(h + 1) * 65],
                                rhs=pts[j][:, half * LC:(half + 1) * LC],
                                start=(mt == 0),
                                stop=(mt == MT - 1),
                            )

                    # normalize: rows 0..63 = O^T, row 64 = denominators
                    rc = rcpp.tile([128, LC], f32)
                    nc.vector.reciprocal(rc[64:65, :], po[64:65, :])
                    # partition_broadcast reads physical partition 0 on HW;
                    # stage the reciprocal row there via a small SBUF DMA
                    rc0 = rcpp.tile([1, LC], f32, name="rc0")
                    nc.sync.dma_start(out=rc0[0:1, :], in_=rc[64:65, :])
                    rb = rcpp.tile([64, LC], f32)
                    nc.gpsimd.partition_broadcast(rb[:], rc0[0:1, :])
                    nt = nrmp.tile([64, LC], f32r)
                    nc.vector.tensor_mul(nt[:], po[0:64, :], rb[:])
                    nc.sync.dma_start(
                        out=ot[h // 2][po_off:po_off + 64, ls], in_=nt[:]
                    )

            # ---- P4: out = O @ woT (direct PSUM -> DRAM DMA) ----
            if "p4" not in phases:
                if "p1" in phases:
                    nc.sync.dma_start(out=out[128:256, 0:512],
                                      in_=qp[0][:, 0:512].bitcast(f32))
                if "av" in phases:
                    nc.sync.dma_start(out=out[0:128, 0:512],
                                      in_=ot[0][:, 0:512].bitcast(f32))
                elif "scores" in phases and pts:
                    nc.sync.dma_start(out=out[0:128, 0:512],
                                      in_=pts[0][:, 0:512].bitcast(f32))
                if "vt" in phases:
                    nc.sync.dma_start(out=out[256:384, 0:260],
                                      in_=vt[0][:].bitcast(f32))
            for lt in (range(MT) if "p4" in phases else []):
                for ec in range(NEC):
                    ps = psmm.tile([128, 512], f32, name="ps4", tag="ps")
                    for t in range(2):
                        nc.tensor.matmul(
                            ps[:],
                            lhsT=ot[t][:, lt * 128:(lt + 1) * 128],
                            rhs=wo_sb[t][:, ec * 512:(ec + 1) * 512],
                            start=(t == 0),
                            stop=(t == 1),
                        )
                    st = stgp.tile([128, 512], f32)
                    nc.vector.tensor_copy(st[:], ps[:])
                    nc.sync.dma_start(
                        out=out[lt * 128:(lt + 1) * 128, ec * 512:(ec + 1) * 512],
                        in_=st[:],
                    )
            _p34.close()

    nc.compile()
    return nc


def _prep_in_maps(x, w_qkv, b_qkv, w_o):
    xT = [np.ascontiguousarray(x[b].T) for b in range(B)]
    in_maps = []
    for core in range(8):
        b, g = divmod(core, 4)
        qs, ks, vs = g * G, D + g * G, 2 * D + g * G
        wqkvT = np.ascontiguousarray(
            np.concatenate(
                [w_qkv[qs:qs + G], w_qkv[ks:ks + G], w_qkv[vs:vs + G]], axis=0
            ).T
        )
        bqk_m = np.ascontiguousarray(
            np.concatenate([b_qkv[qs:qs + G], b_qkv[ks:ks + G]]).reshape(4, 128).T
        )
        woT = np.ascontiguousarray(w_o[:, g * G:(g + 1) * G].T)
        in_maps.append({"xT": xT[b], "wqkvT": wqkvT, "bqk": bqk_m, "woT": woT})
    return in_maps


def kernel(x, w_qkv, b_qkv, w_o, b_o):
    from concourse.bass_utils import run_bass_kernel_spmd

    x = np.asarray(x, dtype=np.float32)
    w_qkv = np.asarray(w_qkv, dtype=np.float32)
    b_qkv = np.asarray(b_qkv, dtype=np.float32)
    w_o = np.asarray(w_o, dtype=np.float32)
    b_o = np.asarray(b_o, dtype=np.float32)

    if "nc" not in _CACHE:
        _CACHE["nc"] = _build()
    nc = _CACHE["nc"]

    in_maps = _prep_in_maps(x, w_qkv, b_qkv, w_o)
    res = run_bass_kernel_spmd(nc, in_maps, list(range(8)))
    partial = np.stack([res.results[i]["out"] for i in range(8)])  # [8, L, D]

    const = w_o @ b_qkv[2 * D:] + b_o  # [D]
    out = partial.reshape(B, 4, L, D).sum(axis=1) + x + const[None, None, :]
    return out.astype(np.float32)



# revision 2
# speedup vs baseline: 1.4625x; 1.4625x over previous
"""Trainium2 Bass kernel for a transformer attention block (BasicBlock), v4.

Reference computation (B=2, L=2048, D=1024, H=16, C=64):
    qkv = x @ w_qkv.T + b_qkv ; q,k,v = split(qkv)
    attn = softmax((q @ k.T) / sqrt(D)) ; heads = attn @ v
    out  = heads @ w_o.T + b_o + x
Sharding: 8 cores = 2 batches x 4 head-groups (4 heads each).

v4 design:
- all matmul operands bf16 (PSUM accumulation fp32)
- input DMA in consumption order, alternating over the two HWDGE queues
  (SP, ACT); kz zero-fill on DVE so the Pool queue stays clear
- P1 (QK^T proj) and P2 (V proj) interleaved per l-chunk behind the
  arriving x chunks
- attention lc-major; per (h, lc) the 8 score PSUM tiles alternate
  eviction between ACT (exact exp, j even) and DVE (Schraudolph fast
  exp in bf16-bits-via-int16, j odd); pssc has 3 PSUM bufs so score
  fills never wait on evictions
- P4 (out proj) software-pipelined: one 128-row block of the previous
  lc per head slot, PSUM from the same pssc pool ([128,1024] pairs both
  e-chunks), single ACT eviction to bf16, single row-block DMA; host
  sums the bf16 partials in fp32
- the normalization multiply nt(h) is emitted mid-way through head h+1's
  evictions so the DVE never idles waiting for the reciprocal broadcast
"""

import sys

if "/opt/trn_rl_repo" not in sys.path:
    sys.path.insert(0, "/opt/trn_rl_repo")

import numpy as np

B, L, D, H = 2, 2048, 1024, 16
C = 64
HPC = 4            # heads per core
G = 256            # dims per head group (HPC * C)
SCALE = float(1.0 / np.sqrt(np.float32(D)))

LC = 512           # l-chunk (moving dim)
NLC = L // LC      # 4
MT = L // 128      # 16 m-tiles
DT = D // 128      # 8 d-tiles
NEC = D // 512     # 2 e-chunks for out projection

# Schraudolph fast exp in bf16: exp(s*SCALE) ~= bitcast_bf16(int16(s*A + B))
# A = 128*log2(e)*SCALE ; B = 127*128 - c, c = RMS-optimal shift (~7.42)
SCH_A = float(128.0 * np.log2(np.e) * SCALE)
SCH_B = float(127.0 * 128.0 - 7.4219)

_CACHE = {}

ALL_PHASES = ("p1", "vt", "scores", "exp", "av", "p4")


def _build(reps=1, phases=ALL_PHASES):
    import concourse.mybir as mybir
    import concourse.tile as tile
    from concourse import bacc

    f32 = mybir.dt.float32
    bf16 = mybir.dt.bfloat16
    i16 = mybir.dt.int16
    Exp = mybir.ActivationFunctionType.Exp
    Copy = mybir.ActivationFunctionType.Copy

    nc = bacc.Bacc("TRN2", target_bir_lowering=False, debug=False)

    xT = nc.declare_dram_parameter("xT", [D, L], bf16, isOutput=False)
    # columns: [Q (256) | K (256) | V (256)] of this head group, transposed
    wqkvT = nc.declare_dram_parameter("wqkvT", [D, 3 * G], bf16, isOutput=False)
    bqk = nc.declare_dram_parameter("bqk", [128, 4], f32, isOutput=False)
    woT = nc.declare_dram_parameter("woT", [G, D], bf16, isOutput=False)
    out = nc.declare_dram_parameter("out", [L, D], bf16, isOutput=True)

    with tile.TileContext(nc) as tc:
      for _rep in range(reps):
        with (
            tc.tile_pool(name="const", bufs=1) as constp,
            tc.tile_pool(name="xt", bufs=DT) as xtp,
            tc.tile_pool(name="wqkv", bufs=DT) as wqkvp,
            tc.tile_pool(name="qp", bufs=2) as qpp,
            tc.tile_pool(name="kz", bufs=4) as kzp,
            tc.tile_pool(name="vt", bufs=16) as vtp,
            tc.tile_pool(name="wo", bufs=2) as wop,
            tc.tile_pool(name="ot", bufs=2) as otp,
            tc.tile_pool(name="pt", bufs=12) as ptp,
            tc.tile_pool(name="rcp", bufs=3) as rcpp,
            tc.tile_pool(name="nrm", bufs=3) as nrmp,
            tc.tile_pool(name="stg", bufs=3) as stgp,
        ):
            bqk_sb = constp.tile([128, 4], f32)
            nc.sync.dma_start(out=bqk_sb[:], in_=bqk[:])

            # persistent tiles
            qp = [qpp.tile([128, L], bf16, name="qp", tag="qp") for _ in range(2)]
            kz = [kzp.tile([128, L], bf16, name="kz", tag="kz") for _ in range(HPC)]
            # v[mt]: [128, 4*65]; per head block: [V_h (64 cols) | ones]
            vt = [vtp.tile([128, HPC * 65], bf16, name="vt", tag="vt") for _ in range(MT)]
            ot = [otp.tile([128, L], bf16, name="ot", tag="ot") for _ in range(2)]
            xt = [xtp.tile([128, L], bf16, name="x_sb", tag="x_sb") for _ in range(DT)]
            wq = [wqkvp.tile([128, 3 * G], bf16, name="wqkv_sb", tag="wqkv_sb")
                  for _ in range(DT)]
            wo_sb = [wop.tile([128, D], bf16, name="wo_sb", tag="wo_sb")
                     for _ in range(2)]

            # zero the unused parity halves of kz once (Pool is idle here and
            # nothing else queues on it until the attention phase)
            for h in range(HPC):
                zs = slice(64, 128) if h % 2 == 0 else slice(0, 64)
                nc.gpsimd.memset(kz[h][zs, :], 0.0)

            # ---- input DMA in consumption order over both HWDGE queues ----
            dmaq = [nc.sync, nc.scalar]
            _qi = [0]

            def dma_in(out_ap, in_ap):
                dmaq[_qi[0] % len(dmaq)].dma_start(out=out_ap, in_=in_ap)
                _qi[0] += 1

            # K cols of wqkv and x chunk 0 first, then Q cols, V cols, x 1..3
            for d in range(DT):
                dma_in(wq[d][:, G:2 * G], wqkvT[d * 128:(d + 1) * 128, G:2 * G])
                dma_in(xt[d][:, 0:LC], xT[d * 128:(d + 1) * 128, 0:LC])
            for d in range(DT):
                dma_in(wq[d][:, 0:G], wqkvT[d * 128:(d + 1) * 128, 0:G])
            for d in range(DT):
                dma_in(wq[d][:, 2 * G:3 * G], wqkvT[d * 128:(d + 1) * 128, 2 * G:3 * G])
            for c in range(1, NLC):
                cs = slice(c * LC, (c + 1) * LC)
                for d in range(DT):
                    dma_in(xt[d][:, cs], xT[d * 128:(d + 1) * 128, cs])
            for t in range(2):
                dma_in(wo_sb[t][:], woT[t * 128:(t + 1) * 128, :])

            # ---- P1+P2 interleaved per l-chunk ----
            # t: 0,1 = Q pairs; 2,3 = K pairs (wq cols t*128..); V = cols 2G..3G
            with tc.tile_pool(name="ps_mm", bufs=2, space="PSUM") as psmm:
                for lc in (range(NLC) if "p1" in phases else []):
                    ls = slice(lc * LC, (lc + 1) * LC)
                    for t in [2, 0, 3, 1]:
                        ps = psmm.tile([128, LC], f32, name="ps", tag="ps")
                        for d in range(DT):
                            nc.tensor.matmul(
                                ps[:],
                                lhsT=wq[d][:, t * 128:(t + 1) * 128],
                                rhs=xt[d][:, ls],
                                start=(d == 0),
                                stop=(d == DT - 1),
                            )
                        if t < 2:
                            nc.vector.tensor_scalar_add(
                                qp[t][:, ls], ps[:], bqk_sb[:, t:t + 1]
                            )
                        else:
                            h0 = 2 * (t - 2)
                            nc.vector.tensor_scalar_add(
                                kz[h0][0:64, ls], ps[0:64, :], bqk_sb[0:64, t:t + 1]
                            )
                            nc.vector.tensor_scalar_add(
                                kz[h0 + 1][64:128, ls], ps[64:128, :],
                                bqk_sb[64:128, t:t + 1],
                            )
                    if "vt" not in phases:
                        continue
                    for mt in range(4 * lc, 4 * lc + 4):
                        ps = psmm.tile([128, G], f32, name="ps", tag="ps")
                        for d in range(DT):
                            nc.tensor.matmul(
                                ps[:],
                                lhsT=xt[d][:, mt * 128:(mt + 1) * 128],
                                rhs=wq[d][:, 2 * G:3 * G],
                                start=(d == 0),
                                stop=(d == DT - 1),
                            )
                        v3d = vt[mt][:].rearrange("p (h c) -> p h c", h=HPC)
                        nc.vector.tensor_copy(
                            v3d[:, :, 0:64], ps[:].rearrange("p (h c) -> p h c", h=HPC)
                        )
                        nc.vector.tensor_scalar(
                            v3d[:, :, 64:65], v3d[:, :, 0:1], 0.0, 1.0,
                            mybir.AluOpType.mult, mybir.AluOpType.add,
                        )

            # ---- attention (lc-major), P4 of previous lc pipelined in ----
            with (
                tc.tile_pool(name="ps_sc", bufs=3, space="PSUM") as pssc,
                tc.tile_pool(name="ps_o", bufs=2, space="PSUM") as pso,
            ):

                def p4_block(lt, drain=False):
                    """Out-projection for one 128-row block: both e-chunks
                    into one [128,1024] PSUM tile, one eviction, then the
                    row-block DMA split so one DMA engine's bandwidth never
                    gates the drain. Pipelined blocks keep all issue cost on
                    the SP queue (the ACT queue's issue time would pace the
                    exp evictions); drain blocks fan out over both queues
                    and both eviction engines since compute is over."""
                    ps = pssc.tile([128, 2 * LC], f32, name="ps4", tag="ps")
                    for ec in range(NEC):
                        for t in range(2):
                            nc.tensor.matmul(
                                ps[:, ec * 512:(ec + 1) * 512],
                                lhsT=ot[t][:, lt * 128:(lt + 1) * 128],
                                rhs=wo_sb[t][:, ec * 512:(ec + 1) * 512],
                                start=(t == 0),
                                stop=(t == 1),
                            )
                    st = stgp.tile([128, 2 * LC], bf16)
                    if drain and lt % 2 == 1:
                        nc.vector.tensor_copy(st[:], ps[:])
                    else:
                        nc.scalar.activation(st[:], ps[:], Copy)
                    nsplit = 4 if drain else 2
                    for q in range(nsplit):
                        qs = slice(q * (2 * LC // nsplit), (q + 1) * (2 * LC // nsplit))
                        eng = dmaq[q % 2] if drain else nc.sync
                        eng.dma_start(
                            out=out[lt * 128:(lt + 1) * 128, qs], in_=st[:, qs]
                        )

                def norm_head(lc, h, po):
                    """Emit the normalization for head h of chunk lc; the
                    final multiply is returned as a closure so the caller can
                    emit it later in the DVE stream."""
                    ls = slice(lc * LC, (lc + 1) * LC)
                    po_off = (h % 2) * 64
                    rc = rcpp.tile([128, LC], f32)
                    nc.vector.reciprocal(rc[64:65, :], po[64:65, :])
                    # partition_broadcast reads physical partition 0 on HW;
                    # stage the reciprocal row there via a small SBUF DMA
                    rc0 = rcpp.tile([1, LC], f32, name="rc0")
                    nc.sync.dma_start(out=rc0[0:1, :], in_=rc[64:65, :])
                    rb = rcpp.tile([64, LC], f32)
                    nc.gpsimd.partition_broadcast(rb[:], rc0[0:1, :])

                    def emit_nt():
                        nt = nrmp.tile([64, LC], bf16)
                        nc.vector.tensor_mul(nt[:], po[0:64, :], rb[:])
                        nc.sync.dma_start(
                            out=ot[h // 2][po_off:po_off + 64, ls], in_=nt[:]
                        )

                    return emit_nt

                pending_nt = None    # deferred normalization multiply
                pending_p4 = []      # row blocks of the previous lc
                pts = None
                for lc in (range(NLC) if "scores" in phases else []):
                    ls = slice(lc * LC, (lc + 1) * LC)
                    for h in range(HPC):
                        pts = []
                        for j in range(MT // 2):
                            ps = pssc.tile([128, 2 * LC], f32, name="ps", tag="ps")
                            for half in range(2):
                                mt = 2 * j + half
                                nc.tensor.matmul(
                                    ps[:, half * LC:(half + 1) * LC],
                                    lhsT=kz[h][:, mt * 128:(mt + 1) * 128],
                                    rhs=qp[h // 2][:, ls],
                                    start=True,
                                    stop=True,
                                )
                            ptile = ptp.tile([128, 2 * LC], bf16)
                            if "exp" in phases:
                                if j % 2 == 0:
                                    nc.scalar.activation(
                                        ptile[:], ps[:], Exp, scale=SCALE
                                    )
                                else:
                                    nc.vector.tensor_scalar(
                                        ptile[:].bitcast(i16), ps[:], SCH_A, SCH_B,
                                        mybir.AluOpType.mult, mybir.AluOpType.add,
                                    )
                            else:
                                nc.vector.tensor_copy(ptile[:, 0:8], ps[:, 0:8])
                            pts.append(ptile)
                            if j == 3 and pending_nt is not None:
                                # previous head's normalization: its broadcast
                                # is ready by now, so this never stalls DVE
                                pending_nt()
                                pending_nt = None

                        if "av" not in phases:
                            continue
                        po = pso.tile([65, LC], f32, name="po", tag="po")
                        for j in range(MT // 2):
                            for half in range(2):
                                mt = 2 * j + half
                                nc.tensor.matmul(
                                    po[:],
                                    lhsT=vt[mt][:, h * 65:(h + 1) * 65],
                                    rhs=pts[j][:, half * LC:(half + 1) * LC],
                                    start=(mt == 0),
                                    stop=(mt == MT - 1),
                                )
                        pending_nt = norm_head(lc, h, po)

                        # one row block of the previous lc's out-projection
                        if pending_p4 and "p4" in phases:
                            p4_block(pending_p4.pop(0))

                    if "av" in phases:
                        pending_p4.extend(range(4 * lc, 4 * lc + 4))

                if pending_nt is not None:
                    pending_nt()
                    pending_nt = None
                # drain: last lc's out-projection, evictions on both engines
                if "p4" in phases:
                    for lt in pending_p4:
                        p4_block(lt, drain=True)
                pending_p4 = []

                # debug outputs for phase-subset builds
                if "p4" not in phases or "av" not in phases:
                    if "p1" in phases:
                        nc.sync.dma_start(out=out[128:256, 0:1024],
                                          in_=qp[0][:, 0:1024])
                    if "av" in phases:
                        nc.sync.dma_start(out=out[0:128, 0:1024],
                                          in_=ot[0][:, 0:1024])
                    elif "scores" in phases and pts:
                        nc.sync.dma_start(out=out[0:128, 0:1024],
                                          in_=pts[0][:, 0:1024])
                    if "vt" in phases:
                        nc.sync.dma_start(out=out[256:384, 0:260],
                                          in_=vt[0][:])

    nc.compile()
    return nc


def _prep_in_maps(x, w_qkv, b_qkv, w_o):
    import ml_dtypes

    bf16 = ml_dtypes.bfloat16
    xT = [np.ascontiguousarray(x[b].T).astype(bf16) for b in range(B)]
    in_maps = []
    for core in range(8):
        b, g = divmod(core, 4)
        qs, ks, vs = g * G, D + g * G, 2 * D + g * G
        wqkvT = np.ascontiguousarray(
            np.concatenate(
                [w_qkv[qs:qs + G], w_qkv[ks:ks + G], w_qkv[vs:vs + G]], axis=0
            ).T
        ).astype(bf16)
        bqk_m = np.ascontiguousarray(
            np.concatenate([b_qkv[qs:qs + G], b_qkv[ks:ks + G]]).reshape(4, 128).T
        )
        woT = np.ascontiguousarray(w_o[:, g * G:(g + 1) * G].T).astype(bf16)
        in_maps.append({"xT": xT[b], "wqkvT": wqkvT, "bqk": bqk_m, "woT": woT})
    return in_maps


def kernel(x, w_qkv, b_qkv, w_o, b_o):
    from concourse.bass_utils import run_bass_kernel_spmd

    x = np.asarray(x, dtype=np.float32)
    w_qkv = np.asarray(w_qkv, dtype=np.float32)
    b_qkv = np.asarray(b_qkv, dtype=np.float32)
    w_o = np.asarray(w_o, dtype=np.float32)
    b_o = np.asarray(b_o, dtype=np.float32)

    if "nc" not in _CACHE:
        _CACHE["nc"] = _build()
    nc = _CACHE["nc"]

    in_maps = _prep_in_maps(x, w_qkv, b_qkv, w_o)
    res = run_bass_kernel_spmd(nc, in_maps, list(range(8)))
    partial = np.stack(
        [np.asarray(res.results[i]["out"], dtype=np.float32) for i in range(8)]
    )  # [8, L, D]

    const = w_o @ b_qkv[2 * D:] + b_o  # [D]
    out = partial.reshape(B, 4, L, D).sum(axis=1) + x + const[None, None, :]
    return out.astype(np.float32)


# revision 4
# speedup vs baseline: 1.5323x; 1.0477x over previous
"""Trainium2 Bass kernel for a transformer attention block (BasicBlock), v4.

Reference computation (B=2, L=2048, D=1024, H=16, C=64):
    qkv = x @ w_qkv.T + b_qkv ; q,k,v = split(qkv)
    attn = softmax((q @ k.T) / sqrt(D)) ; heads = attn @ v
    out  = heads @ w_o.T + b_o + x
Sharding: 8 cores = 2 batches x 4 head-groups (4 heads each).

v4 design:
- all matmul operands bf16 (PSUM accumulation fp32)
- input DMA in consumption order, alternating over the two HWDGE queues
  (SP, ACT); kz zero-fill on DVE so the Pool queue stays clear
- P1 (QK^T proj) and P2 (V proj) interleaved per l-chunk behind the
  arriving x chunks
- attention lc-major; per (h, lc) the 8 score PSUM tiles alternate
  eviction between ACT (exact exp, j even) and DVE (Schraudolph fast
  exp in bf16-bits-via-int16, j odd); pssc has 3 PSUM bufs so score
  fills never wait on evictions
- P4 (out proj) software-pipelined: one 128-row block of the previous
  lc per head slot, PSUM from the same pssc pool ([128,1024] pairs both
  e-chunks), single ACT eviction to bf16, single row-block DMA; host
  sums the bf16 partials in fp32
- the normalization multiply nt(h) is emitted mid-way through head h+1's
  evictions so the DVE never idles waiting for the reciprocal broadcast
"""

import sys

if "/opt/trn_rl_repo" not in sys.path:
    sys.path.insert(0, "/opt/trn_rl_repo")

import numpy as np

B, L, D, H = 2, 2048, 1024, 16
C = 64
HPC = 4            # heads per core
G = 256            # dims per head group (HPC * C)
SCALE = float(1.0 / np.sqrt(np.float32(D)))

LC = 512           # l-chunk (moving dim)
NLC = L // LC      # 4
MT = L // 128      # 16 m-tiles
DT = D // 128      # 8 d-tiles
NEC = D // 512     # 2 e-chunks for out projection

# Schraudolph fast exp in bf16: exp(s*SCALE) ~= bitcast_bf16(int16(s*A + B))
# A = 128*log2(e)*SCALE ; B = 127*128 - c, c = RMS-optimal shift (~7.42)
SCH_A = float(128.0 * np.log2(np.e) * SCALE)
SCH_B = float(127.0 * 128.0 - 7.4219)
# same trick into fp8e4m3 bits via int8: 3 mantissa bits, bias 7
SCH_A8 = float(8.0 * np.log2(np.e) * SCALE)
SCH_B8 = float(7.0 * 8.0 - 0.4639)

_CACHE = {}

ALL_PHASES = ("p1", "vt", "scores", "exp", "av", "p4")


def _build(reps=1, phases=ALL_PHASES):
    import concourse.mybir as mybir
    import concourse.tile as tile
    from concourse import bacc

    f32 = mybir.dt.float32
    bf16 = mybir.dt.bfloat16
    i16 = mybir.dt.int16
    i8 = mybir.dt.int8
    fp8 = mybir.dt.float8e4
    DR = mybir.MatmulPerfMode.DoubleRow
    Exp = mybir.ActivationFunctionType.Exp
    Copy = mybir.ActivationFunctionType.Copy

    nc = bacc.Bacc("TRN2", target_bir_lowering=False, debug=False)

    xT = nc.declare_dram_parameter("xT", [D, L], bf16, isOutput=False)
    # columns: [Q (256) | K (256) | V (256)] of this head group, transposed
    wqkvT = nc.declare_dram_parameter("wqkvT", [D, 3 * G], bf16, isOutput=False)
    bqk = nc.declare_dram_parameter("bqk", [128, 4], f32, isOutput=False)
    woT = nc.declare_dram_parameter("woT", [G, D], bf16, isOutput=False)
    out = nc.declare_dram_parameter("out", [L, D], bf16, isOutput=True)

    with tile.TileContext(nc) as tc:
      for _rep in range(reps):
        with (
            tc.tile_pool(name="const", bufs=1) as constp,
            tc.tile_pool(name="xt", bufs=DT) as xtp,
            tc.tile_pool(name="wqkv", bufs=DT) as wqkvp,
            tc.tile_pool(name="qp", bufs=2) as qpp,
            tc.tile_pool(name="kz", bufs=4) as kzp,
            tc.tile_pool(name="vt", bufs=16) as vtp,
            tc.tile_pool(name="wo", bufs=2) as wop,
            tc.tile_pool(name="ot", bufs=2) as otp,
            tc.tile_pool(name="pt", bufs=12) as ptp,
            tc.tile_pool(name="rcp", bufs=3) as rcpp,
            tc.tile_pool(name="nrm", bufs=3) as nrmp,
            tc.tile_pool(name="stg", bufs=3) as stgp,
        ):
            bqk_sb = constp.tile([128, 4], f32)
            nc.sync.dma_start(out=bqk_sb[:], in_=bqk[:])

            # persistent tiles
            qp = [qpp.tile([128, L], bf16, name="qp", tag="qp") for _ in range(2)]
            kz = [kzp.tile([128, L], bf16, name="kz", tag="kz") for _ in range(HPC)]
            # v[j]: fp8 pair tile [128, (i, h, m)] for mt pair (2j, 2j+1);
            # per (i, head) block: [V_h (64) | ones | pad to 72] — the
            # DoubleRow stationary layout for the AV matmul; the i-group
            # stride (4*72=288) must be a multiple of 32 per the dual-fp8
            # ldweights ISA restriction, hence the pad
            vt = [vtp.tile([128, 2 * HPC * 72], fp8, name="vt", tag="vt")
                  for _ in range(MT // 2)]
            ot = [otp.tile([128, L], bf16, name="ot", tag="ot") for _ in range(2)]
            xt = [xtp.tile([128, L], bf16, name="x_sb", tag="x_sb") for _ in range(DT)]
            wq = [wqkvp.tile([128, 3 * G], bf16, name="wqkv_sb", tag="wqkv_sb")
                  for _ in range(DT)]
            wo_sb = [wop.tile([128, D], bf16, name="wo_sb", tag="wo_sb")
                     for _ in range(2)]

            # zero the unused parity halves of kz once (Pool is idle here and
            # nothing else queues on it until the attention phase)
            for h in range(HPC):
                zs = slice(64, 128) if h % 2 == 0 else slice(0, 64)
                nc.gpsimd.memset(kz[h][zs, :], 0.0)

            # ---- input DMA in consumption order over both HWDGE queues ----
            dmaq = [nc.sync, nc.scalar]
            _qi = [0]

            def dma_in(out_ap, in_ap):
                dmaq[_qi[0] % len(dmaq)].dma_start(out=out_ap, in_=in_ap)
                _qi[0] += 1

            # K cols of wqkv and x chunk 0 first, then Q cols, V cols, x 1..3
            for d in range(DT):
                dma_in(wq[d][:, G:2 * G], wqkvT[d * 128:(d + 1) * 128, G:2 * G])
                dma_in(xt[d][:, 0:LC], xT[d * 128:(d + 1) * 128, 0:LC])
            for d in range(DT):
                dma_in(wq[d][:, 0:G], wqkvT[d * 128:(d + 1) * 128, 0:G])
            for d in range(DT):
                dma_in(wq[d][:, 2 * G:3 * G], wqkvT[d * 128:(d + 1) * 128, 2 * G:3 * G])
            for c in range(1, NLC):
                cs = slice(c * LC, (c + 1) * LC)
                for d in range(DT):
                    dma_in(xt[d][:, cs], xT[d * 128:(d + 1) * 128, cs])
            for t in range(2):
                dma_in(wo_sb[t][:], woT[t * 128:(t + 1) * 128, :])

            # ---- P1+P2 interleaved per l-chunk ----
            # t: 0,1 = Q pairs; 2,3 = K pairs (wq cols t*128..); V = cols 2G..3G
            with tc.tile_pool(name="ps_mm", bufs=2, space="PSUM") as psmm:
                for lc in (range(NLC) if "p1" in phases else []):
                    ls = slice(lc * LC, (lc + 1) * LC)
                    for t in [2, 0, 3, 1]:
                        ps = psmm.tile([128, LC], f32, name="ps", tag="ps")
                        for d in range(DT):
                            nc.tensor.matmul(
                                ps[:],
                                lhsT=wq[d][:, t * 128:(t + 1) * 128],
                                rhs=xt[d][:, ls],
                                start=(d == 0),
                                stop=(d == DT - 1),
                            )
                        if t < 2:
                            nc.vector.tensor_scalar_add(
                                qp[t][:, ls], ps[:], bqk_sb[:, t:t + 1]
                            )
                        else:
                            h0 = 2 * (t - 2)
                            nc.vector.tensor_scalar_add(
                                kz[h0][0:64, ls], ps[0:64, :], bqk_sb[0:64, t:t + 1]
                            )
                            nc.vector.tensor_scalar_add(
                                kz[h0 + 1][64:128, ls], ps[64:128, :],
                                bqk_sb[64:128, t:t + 1],
                            )
                    if "vt" not in phases:
                        continue
                    for mt in range(4 * lc, 4 * lc + 4):
                        ps = psmm.tile([128, G], f32, name="ps", tag="ps")
                        for d in range(DT):
                            nc.tensor.matmul(
                                ps[:],
                                lhsT=xt[d][:, mt * 128:(mt + 1) * 128],
                                rhs=wq[d][:, 2 * G:3 * G],
                                start=(d == 0),
                                stop=(d == DT - 1),
                            )
                        v4d = vt[mt // 2][:].rearrange(
                            "p (i h c) -> p i h c", i=2, h=HPC
                        )[:, mt % 2]
                        nc.vector.tensor_copy(
                            v4d[:, :, 0:64], ps[:].rearrange("p (h c) -> p h c", h=HPC)
                        )
                        nc.vector.tensor_scalar(
                            v4d[:, :, 64:65], v4d[:, :, 0:1], 0.0, 1.0,
                            mybir.AluOpType.mult, mybir.AluOpType.add,
                        )

            # ---- attention (lc-major), P4 of previous lc pipelined in ----
            with (
                tc.tile_pool(name="ps_sc", bufs=3, space="PSUM") as pssc,
                tc.tile_pool(name="ps_o", bufs=2, space="PSUM") as pso,
            ):

                def p4_block(lt, drain=False):
                    """Out-projection for one 128-row block: both e-chunks
                    into one [128,1024] PSUM tile, one eviction, then the
                    row-block DMA split so one DMA engine's bandwidth never
                    gates the drain. Pipelined blocks keep all issue cost on
                    the SP queue (the ACT queue's issue time would pace the
                    exp evictions); drain blocks fan out over both queues
                    and both eviction engines since compute is over."""
                    ps = pssc.tile([128, 2 * LC], f32, name="ps4", tag="ps")
                    for ec in range(NEC):
                        for t in range(2):
                            nc.tensor.matmul(
                                ps[:, ec * 512:(ec + 1) * 512],
                                lhsT=ot[t][:, lt * 128:(lt + 1) * 128],
                                rhs=wo_sb[t][:, ec * 512:(ec + 1) * 512],
                                start=(t == 0),
                                stop=(t == 1),
                            )
                    st = stgp.tile([128, 2 * LC], bf16)
                    if drain and lt % 2 == 1:
                        nc.vector.tensor_copy(st[:], ps[:])
                    else:
                        nc.scalar.activation(st[:], ps[:], Copy)
                    nsplit = 4 if drain else 2
                    for q in range(nsplit):
                        qs = slice(q * (2 * LC // nsplit), (q + 1) * (2 * LC // nsplit))
                        eng = dmaq[q % 2] if drain else nc.sync
                        eng.dma_start(
                            out=out[lt * 128:(lt + 1) * 128, qs], in_=st[:, qs]
                        )

                def norm_head(lc, h, po):
                    """Emit the normalization for head h of chunk lc; the
                    final multiply is returned as a closure so the caller can
                    emit it later in the DVE stream."""
                    ls = slice(lc * LC, (lc + 1) * LC)
                    po_off = (h % 2) * 64
                    rc = rcpp.tile([128, LC], f32)
                    nc.vector.reciprocal(rc[64:65, :], po[64:65, :])
                    # partition_broadcast reads physical partition 0 on HW;
                    # stage the reciprocal row there via a small SBUF DMA
                    rc0 = rcpp.tile([1, LC], f32, name="rc0")
                    nc.sync.dma_start(out=rc0[0:1, :], in_=rc[64:65, :])
                    rb = rcpp.tile([64, LC], f32)
                    nc.gpsimd.partition_broadcast(rb[:], rc0[0:1, :])

                    def emit_nt():
                        nt = nrmp.tile([64, LC], bf16)
                        nc.vector.tensor_mul(nt[:], po[0:64, :], rb[:])
                        nc.sync.dma_start(
                            out=ot[h // 2][po_off:po_off + 64, ls], in_=nt[:]
                        )

                    return emit_nt

                pending_nt = None    # deferred normalization multiply
                pending_p4 = []      # row blocks of the previous lc
                pts = None
                for lc in (range(NLC) if "scores" in phases else []):
                    ls = slice(lc * LC, (lc + 1) * LC)
                    for h in range(HPC):
                        pts = []
                        for j in range(MT // 2):
                            ps = pssc.tile([128, 2 * LC], f32, name="ps", tag="ps")
                            for half in range(2):
                                mt = 2 * j + half
                                nc.tensor.matmul(
                                    ps[:, half * LC:(half + 1) * LC],
                                    lhsT=kz[h][:, mt * 128:(mt + 1) * 128],
                                    rhs=qp[h // 2][:, ls],
                                    start=True,
                                    stop=True,
                                )
                            ptile = ptp.tile([128, 2 * LC], fp8)
                            if "exp" in phases:
                                if j % 2 == 0:
                                    nc.scalar.activation(
                                        ptile[:], ps[:], Exp, scale=SCALE
                                    )
                                else:
                                    nc.vector.tensor_scalar(
                                        ptile[:].bitcast(i8), ps[:], SCH_A8, SCH_B8,
                                        mybir.AluOpType.mult, mybir.AluOpType.add,
                                    )
                            else:
                                nc.vector.tensor_copy(ptile[:, 0:8], ps[:, 0:8])
                            pts.append(ptile)
                            if j == 3 and pending_nt is not None:
                                # previous head's normalization: its broadcast
                                # is ready by now, so this never stalls DVE
                                pending_nt()
                                pending_nt = None

                        if "av" not in phases:
                            continue
                        po = pso.tile([65, LC], f32, name="po", tag="po")
                        for j in range(MT // 2):
                            nc.tensor.matmul(
                                po[:],
                                lhsT=vt[j][:].rearrange(
                                    "p (i h c) -> p i h c", i=2, h=HPC
                                )[:, :, h, 0:65],
                                rhs=pts[j][:].rearrange("p (i q) -> p i q", i=2),
                                start=(j == 0),
                                stop=(j == MT // 2 - 1),
                                perf_mode=DR,
                            )
                        pending_nt = norm_head(lc, h, po)

                        # one row block of the previous lc's out-projection
                        if pending_p4 and "p4" in phases:
                            p4_block(pending_p4.pop(0))

                    if "av" in phases:
                        pending_p4.extend(range(4 * lc, 4 * lc + 4))

                if pending_nt is not None:
                    pending_nt()
                    pending_nt = None
                # drain: last lc's out-projection, evictions on both engines
                if "p4" in phases:
                    for lt in pending_p4:
                        p4_block(lt, drain=True)
                pending_p4 = []

                # debug outputs for phase-subset builds
                if "p4" not in phases or "av" not in phases:
                    if "p1" in phases:
                        nc.sync.dma_start(out=out[128:256, 0:1024],
                                          in_=qp[0][:, 0:1024])
                    if "av" in phases:
                        nc.sync.dma_start(out=out[0:128, 0:1024],
                                          in_=ot[0][:, 0:1024])
                    elif "scores" in phases and pts:
                        nc.sync.dma_start(out=out[0:128, 0:1024],
                                          in_=pts[0][:, 0:1024])
                    if "vt" in phases:
                        nc.sync.dma_start(out=out[256:384, 0:288],
                                          in_=vt[0][:].bitcast(bf16))

    nc.compile()
    return nc


def _prep_in_maps(x, w_qkv, b_qkv, w_o):
    import ml_dtypes

    bf16 = ml_dtypes.bfloat16
    xT = [np.ascontiguousarray(x[b].T).astype(bf16) for b in range(B)]
    in_maps = []
    for core in range(8):
        b, g = divmod(core, 4)
        qs, ks, vs = g * G, D + g * G, 2 * D + g * G
        wqkvT = np.ascontiguousarray(
            np.concatenate(
                [w_qkv[qs:qs + G], w_qkv[ks:ks + G], w_qkv[vs:vs + G]], axis=0
            ).T
        ).astype(bf16)
        bqk_m = np.ascontiguousarray(
            np.concatenate([b_qkv[qs:qs + G], b_qkv[ks:ks + G]]).reshape(4, 128).T
        )
        woT = np.ascontiguousarray(w_o[:, g * G:(g + 1) * G].T).astype(bf16)
        in_maps.append({"xT": xT[b], "wqkvT": wqkvT, "bqk": bqk_m, "woT": woT})
    return in_maps


def kernel(x, w_qkv, b_qkv, w_o, b_o):
    from concourse.bass_utils import run_bass_kernel_spmd

    x = np.asarray(x, dtype=np.float32)
    w_qkv = np.asarray(w_qkv, dtype=np.float32)
    b_qkv = np.asarray(b_qkv, dtype=np.float32)
    w_o = np.asarray(w_o, dtype=np.float32)
    b_o = np.asarray(b_o, dtype=np.float32)

    if "nc" not in _CACHE:
        _CACHE["nc"] = _build()
    nc = _CACHE["nc"]

    in_maps = _prep_in_maps(x, w_qkv, b_qkv, w_o)
    res = run_bass_kernel_spmd(nc, in_maps, list(range(8)))
    partial = np.stack(
        [np.asarray(res.results[i]["out"], dtype=np.float32) for i in range(8)]
    )  # [8, L, D]

    const = w_o @ b_qkv[2 * D:] + b_o  # [D]
    out = partial.reshape(B, 4, L, D).sum(axis=1) + x + const[None, None, :]
    return out.astype(np.float32)


# revision 5
# speedup vs baseline: 1.5755x; 1.0282x over previous
"""Trainium2 Bass kernel for a transformer attention block (BasicBlock), v6.

Reference computation (B=2, L=2048, D=1024, H=16, C=64):
    qkv = x @ w_qkv.T + b_qkv ; q,k,v = split(qkv)
    attn = softmax((q @ k.T) / sqrt(D)) ; heads = attn @ v
    out  = heads @ w_o.T + b_o + x
Sharding: 8 cores = 2 batches x 4 head-groups (4 heads each).

v6 design (on top of v5):
- ALL matmuls except scores run in fp8e4m3 DoubleRow (2 K-groups per
  pass): P1/P2 contract d in pairs from fp8 pair tiles of x and w_qkv,
  AV contracts key-block pairs, P4 contracts the o-dim pair.
- weights are scaled x32 on the host so fp8e4m3 never hits subnormals;
  the scale folds into the exp scale (/32^2) and the host divides the
  partials by 32^2.
- scores stay bf16 (their eviction, not the PE, paces the attention
  phase, so fp8 scores would buy nothing).
- per (h, lc): 8 score PSUM tiles evicted alternately by ACT (exact
  exp -> fp8) and DVE (Schraudolph fp8-bits-via-int8); V eviction on
  ACT; Q/K bias-evictions on DVE; vt 'ones' columns are memset once.
"""

import sys

if "/opt/trn_rl_repo" not in sys.path:
    sys.path.insert(0, "/opt/trn_rl_repo")

import numpy as np

B, L, D, H = 2, 2048, 1024, 16
C = 64
HPC = 4            # heads per core
G = 256            # dims per head group (HPC * C)
SW = 32.0          # host-side fp8 weight scale (both w_qkv and w_o)
SCALE = float(1.0 / np.sqrt(np.float32(D)) / (SW * SW))  # exp scale on device

LC = 512           # l-chunk (moving dim)
NLC = L // LC      # 4
MT = L // 128      # 16 m-tiles
DT = D // 128      # 8 d-tiles
DP = DT // 2       # 4 d-pairs (DoubleRow)
NEC = D // 512     # 2 e-chunks for out projection
VW = 72            # padded per-(i,head) V block width (i-stride 4*72 % 32 == 0)

# Schraudolph fast exp into fp8e4m3 bits via int8: 3 mantissa bits, bias 7
SCH_A8 = float(8.0 * np.log2(np.e) * SCALE)
SCH_B8 = float(7.0 * 8.0 - 0.4639)

_CACHE = {}

ALL_PHASES = ("p1", "vt", "scores", "exp", "av", "p4")


def _build(reps=1, phases=ALL_PHASES):
    import concourse.mybir as mybir
    import concourse.tile as tile
    from concourse import bacc

    f32 = mybir.dt.float32
    bf16 = mybir.dt.bfloat16
    i8 = mybir.dt.int8
    fp8 = mybir.dt.float8e4
    DR = mybir.MatmulPerfMode.DoubleRow
    Exp = mybir.ActivationFunctionType.Exp
    Copy = mybir.ActivationFunctionType.Copy

    nc = bacc.Bacc("TRN2", target_bir_lowering=False, debug=False)

    xT = nc.declare_dram_parameter("xT", [D, L], fp8, isOutput=False)
    # columns: [Q (256) | K (256) | V (256)] of this head group, transposed
    wqkvT = nc.declare_dram_parameter("wqkvT", [D, 3 * G], fp8, isOutput=False)
    bqk = nc.declare_dram_parameter("bqk", [128, 4], f32, isOutput=False)
    woT = nc.declare_dram_parameter("woT", [G, D], fp8, isOutput=False)
    out = nc.declare_dram_parameter("out", [L, D], bf16, isOutput=True)

    with tile.TileContext(nc) as tc:
      for _rep in range(reps):
        with (
            tc.tile_pool(name="const", bufs=1) as constp,
            tc.tile_pool(name="xt", bufs=DP) as xtp,
            tc.tile_pool(name="wqkv", bufs=DP) as wqkvp,
            tc.tile_pool(name="qp", bufs=2) as qpp,
            tc.tile_pool(name="kz", bufs=4) as kzp,
            tc.tile_pool(name="vt", bufs=8) as vtp,
            tc.tile_pool(name="wo", bufs=1) as wop,
            tc.tile_pool(name="ot", bufs=1) as otp,
            tc.tile_pool(name="pt", bufs=12) as ptp,
            tc.tile_pool(name="rcp", bufs=3) as rcpp,
            tc.tile_pool(name="nrm", bufs=3) as nrmp,
            tc.tile_pool(name="stg", bufs=3) as stgp,
        ):
            bqk_sb = constp.tile([128, 4], f32)
            nc.sync.dma_start(out=bqk_sb[:], in_=bqk[:])

            # persistent tiles; x/wq/wo/ot are fp8 DoubleRow pair tiles
            # [p, (i, cols)] pairing adjacent 128-row blocks of the
            # contraction dim
            qp = [qpp.tile([128, L], bf16, name="qp", tag="qp") for _ in range(2)]
            # kp[g]: K^T pair tile for heads (2g, 2g+1) — head 2g in
            # partitions 0:64, head 2g+1 in 64:128 (no zero padding; the
            # score matmuls contract K=64 at the head's partition offset)
            kp = [kzp.tile([128, L], bf16, name="kp", tag="kp") for _ in range(2)]
            # v[j]: [p, (i, h, m)] for key-block pair (2j, 2j+1); per (i, h)
            # block: [V_h (64) | ones | pad to 72]
            vt = [vtp.tile([128, 2 * HPC * VW], fp8, name="vt", tag="vt")
                  for _ in range(MT // 2)]
            ot = otp.tile([128, 2 * L], fp8, name="ot", tag="ot")
            xt = [xtp.tile([128, 2 * L], fp8, name="x_sb", tag="x_sb")
                  for _ in range(DP)]
            wq = [wqkvp.tile([128, 2 * 3 * G], fp8, name="wqkv_sb", tag="wqkv_sb")
                  for _ in range(DP)]
            wo_sb = wop.tile([128, 2 * D], fp8, name="wo_sb", tag="wo_sb")

            def xv(dp):
                return xt[dp][:].rearrange("p (i q) -> p i q", i=2)

            def wv(dp):
                return wq[dp][:].rearrange("p (i e) -> p i e", i=2)

            # set the vt 'ones' columns once (Pool is idle here)
            for j in range(MT // 2):
                v4 = vt[j][:].rearrange("p (i h m) -> p i h m", i=2, h=HPC)
                nc.gpsimd.memset(v4[:, :, :, 64:65], 1.0)

            # ---- input DMA in consumption order over both HWDGE queues ----
            dmaq = [nc.sync, nc.scalar]
            _qi = [0]

            def dma_in(out_ap, in_ap):
                dmaq[_qi[0] % len(dmaq)].dma_start(out=out_ap, in_=in_ap)
                _qi[0] += 1

            def xrow(dp, i):
                return slice((2 * dp + i) * 128, (2 * dp + i + 1) * 128)

            # QK weight cols + x cols 0:1024 first, then V cols, x 1024:2048
            for dp in range(DP):
                for i in range(2):
                    dma_in(wv(dp)[:, i, 0:2 * G], wqkvT[xrow(dp, i), 0:2 * G])
                    dma_in(xv(dp)[:, i, 0:2 * LC], xT[xrow(dp, i), 0:2 * LC])
            for dp in range(DP):
                for i in range(2):
                    dma_in(wv(dp)[:, i, 2 * G:3 * G], wqkvT[xrow(dp, i), 2 * G:3 * G])
            for dp in range(DP):
                for i in range(2):
                    dma_in(xv(dp)[:, i, 2 * LC:4 * LC], xT[xrow(dp, i), 2 * LC:4 * LC])
            wov = wo_sb[:].rearrange("p (i e) -> p i e", i=2)
            for i in range(2):
                dma_in(wov[:, i, :], woT[i * 128:(i + 1) * 128, :])

            # ---- P1+P2 interleaved per l-chunk ----
            # t: 0,1 = Q pairs; 2,3 = K pairs (wq cols t*128..); V = cols 2G..3G
            with tc.tile_pool(name="ps_mm", bufs=2, space="PSUM") as psmm:
                for lc in (range(NLC) if "p1" in phases else []):
                    ls = slice(lc * LC, (lc + 1) * LC)
                    for t in [2, 0, 3, 1]:
                        ps = psmm.tile([128, LC], f32, name="ps", tag="ps")
                        for dp in range(DP):
                            nc.tensor.matmul(
                                ps[:],
                                lhsT=wv(dp)[:, :, t * 128:(t + 1) * 128],
                                rhs=xv(dp)[:, :, ls],
                                start=(dp == 0),
                                stop=(dp == DP - 1),
                                perf_mode=DR,
                            )
                        dst = qp[t] if t < 2 else kp[t - 2]
                        nc.vector.tensor_scalar_add(
                            dst[:, ls], ps[:], bqk_sb[:, t:t + 1]
                        )
                    if "vt" not in phases:
                        continue
                    for mt in range(4 * lc, 4 * lc + 4):
                        ps = psmm.tile([128, G], f32, name="ps", tag="ps")
                        for dp in range(DP):
                            nc.tensor.matmul(
                                ps[:],
                                lhsT=xv(dp)[:, :, mt * 128:(mt + 1) * 128],
                                rhs=wv(dp)[:, :, 2 * G:3 * G],
                                start=(dp == 0),
                                stop=(dp == DP - 1),
                                perf_mode=DR,
                            )
                        v4d = vt[mt // 2][:].rearrange(
                            "p (i h c) -> p i h c", i=2, h=HPC
                        )[:, mt % 2]
                        nc.scalar.activation(
                            v4d[:, :, 0:64],
                            ps[:].rearrange("p (h c) -> p h c", h=HPC),
                            Copy,
                        )

            # ---- attention (lc-major), P4 of previous lc pipelined in ----
            with (
                tc.tile_pool(name="ps_sc", bufs=3, space="PSUM") as pssc,
                tc.tile_pool(name="ps_o", bufs=2, space="PSUM") as pso,
            ):
                otv = ot[:].rearrange("p (i l) -> p i l", i=2)

                def p4_block(lt, drain=False):
                    """Out-projection for one 128-row block: both e-chunks
                    into one [128,1024] PSUM tile via fp8 DoubleRow over the
                    256 o-dims, one eviction, split row-block DMA."""
                    ps = pssc.tile([128, 2 * LC], f32, name="ps4", tag="ps")
                    for ec in range(NEC):
                        nc.tensor.matmul(
                            ps[:, ec * 512:(ec + 1) * 512],
                            lhsT=otv[:, :, lt * 128:(lt + 1) * 128],
                            rhs=wov[:, :, ec * 512:(ec + 1) * 512],
                            start=True,
                            stop=True,
                            perf_mode=DR,
                        )
                    st = stgp.tile([128, 2 * LC], bf16)
                    if drain and lt % 2 == 1:
                        nc.vector.tensor_copy(st[:], ps[:])
                    else:
                        nc.scalar.activation(st[:], ps[:], Copy)
                    nsplit = 4 if drain else 2
                    for q in range(nsplit):
                        qs = slice(q * (2 * LC // nsplit), (q + 1) * (2 * LC // nsplit))
                        eng = dmaq[q % 2] if drain else nc.sync
                        eng.dma_start(
                            out=out[lt * 128:(lt + 1) * 128, qs], in_=st[:, qs]
                        )

                def norm_head(lc, h, po):
                    """Emit the normalization for head h of chunk lc; the
                    final multiply is returned as a closure so the caller can
                    emit it later in the DVE stream."""
                    ls = slice((h // 2) * L + lc * LC, (h // 2) * L + (lc + 1) * LC)
                    po_off = (h % 2) * 64
                    rc = rcpp.tile([128, LC], f32)
                    nc.vector.reciprocal(rc[64:65, :], po[64:65, :])
                    # partition_broadcast reads physical partition 0 on HW;
                    # stage the reciprocal row there via a small SBUF DMA
                    rc0 = rcpp.tile([1, LC], f32, name="rc0")
                    nc.sync.dma_start(out=rc0[0:1, :], in_=rc[64:65, :])
                    rb = rcpp.tile([64, LC], f32)
                    nc.gpsimd.partition_broadcast(rb[:], rc0[0:1, :])

                    def emit_nt():
                        nt = nrmp.tile([64, LC], fp8)
                        nc.vector.tensor_mul(nt[:], po[0:64, :], rb[:])
                        nc.sync.dma_start(
                            out=ot[po_off:po_off + 64, ls], in_=nt[:]
                        )

                    return emit_nt

                pending_nt = None    # deferred normalization multiply
                pending_p4 = []      # row blocks of the previous lc
                pts = None
                for lc in (range(NLC) if "scores" in phases else []):
                    ls = slice(lc * LC, (lc + 1) * LC)
                    for h in range(HPC):
                        hp = slice((h % 2) * 64, (h % 2) * 64 + 64)
                        pts = []
                        for j in range(MT // 2):
                            ps = pssc.tile([128, 2 * LC], f32, name="ps", tag="ps")
                            for half in range(2):
                                mt = 2 * j + half
                                nc.tensor.matmul(
                                    ps[:, half * LC:(half + 1) * LC],
                                    lhsT=kp[h // 2][hp, mt * 128:(mt + 1) * 128],
                                    rhs=qp[h // 2][hp, ls],
                                    start=True,
                                    stop=True,
                                )
                            ptile = ptp.tile([128, 2 * LC], fp8)
                            if "exp" in phases:
                                if j % 2 == 0:
                                    nc.scalar.activation(
                                        ptile[:], ps[:], Exp, scale=SCALE
                                    )
                                else:
                                    nc.vector.tensor_scalar(
                                        ptile[:].bitcast(i8), ps[:], SCH_A8, SCH_B8,
                                        mybir.AluOpType.mult, mybir.AluOpType.add,
                                    )
                            else:
                                nc.vector.tensor_copy(ptile[:, 0:8], ps[:, 0:8])
                            pts.append(ptile)
                            if j == 3 and pending_nt is not None:
                                # previous head's normalization: its broadcast
                                # is ready by now, so this never stalls DVE
                                pending_nt()
                                pending_nt = None

                        if "av" not in phases:
                            continue
                        po = pso.tile([65, LC], f32, name="po", tag="po")
                        for j in range(MT // 2):
                            nc.tensor.matmul(
                                po[:],
                                lhsT=vt[j][:].rearrange(
                                    "p (i h c) -> p i h c", i=2, h=HPC
                                )[:, :, h, 0:65],
                                rhs=pts[j][:].rearrange("p (i q) -> p i q", i=2),
                                start=(j == 0),
                                stop=(j == MT // 2 - 1),
                                perf_mode=DR,
                            )
                        pending_nt = norm_head(lc, h, po)

                        # one row block of the previous lc's out-projection
                        if pending_p4 and "p4" in phases:
                            p4_block(pending_p4.pop(0))

                    if "av" in phases:
                        pending_p4.extend(range(4 * lc, 4 * lc + 4))

                if pending_nt is not None:
                    pending_nt()
                    pending_nt = None
                # drain: last lc's out-projection, evictions on both engines
                if "p4" in phases:
                    for lt in pending_p4:
                        p4_block(lt, drain=True)
                pending_p4 = []

                # debug outputs for phase-subset builds
                if "p4" not in phases or "av" not in phases:
                    if "p1" in phases:
                        nc.sync.dma_start(out=out[128:256, 0:1024],
                                          in_=qp[0][:, 0:1024])
                    if "av" in phases:
                        nc.sync.dma_start(out=out[0:128, 0:1024],
                                          in_=ot[:, 0:1024].bitcast(bf16))
                    elif "scores" in phases and pts:
                        nc.sync.dma_start(out=out[0:128, 0:512],
                                          in_=pts[0][:, 0:1024].bitcast(bf16))
                    if "vt" in phases:
                        nc.sync.dma_start(out=out[256:384, 0:288],
                                          in_=vt[0][:].bitcast(bf16))

    nc.compile()
    return nc


def _prep_in_maps(x, w_qkv, b_qkv, w_o):
    import ml_dtypes

    fp8 = ml_dtypes.float8_e4m3
    xT = [np.ascontiguousarray(x[b].T).astype(fp8) for b in range(B)]
    in_maps = []
    for core in range(8):
        b, g = divmod(core, 4)
        qs, ks, vs = g * G, D + g * G, 2 * D + g * G
        wqkvT = np.ascontiguousarray(
            np.concatenate(
                [w_qkv[qs:qs + G], w_qkv[ks:ks + G], w_qkv[vs:vs + G]], axis=0
            ).T * SW
        ).astype(fp8)
        bqk_m = np.ascontiguousarray(
            np.concatenate([b_qkv[qs:qs + G], b_qkv[ks:ks + G]]).reshape(4, 128).T
            * SW
        ).astype(np.float32)
        woT = np.ascontiguousarray(w_o[:, g * G:(g + 1) * G].T * SW).astype(fp8)
        in_maps.append({"xT": xT[b], "wqkvT": wqkvT, "bqk": bqk_m, "woT": woT})
    return in_maps


def kernel(x, w_qkv, b_qkv, w_o, b_o):
    from concourse.bass_utils import run_bass_kernel_spmd

    x = np.asarray(x, dtype=np.float32)
    w_qkv = np.asarray(w_qkv, dtype=np.float32)
    b_qkv = np.asarray(b_qkv, dtype=np.float32)
    w_o = np.asarray(w_o, dtype=np.float32)
    b_o = np.asarray(b_o, dtype=np.float32)

    if "nc" not in _CACHE:
        _CACHE["nc"] = _build()
    nc = _CACHE["nc"]

    in_maps = _prep_in_maps(x, w_qkv, b_qkv, w_o)
    res = run_bass_kernel_spmd(nc, in_maps, list(range(8)))
    partial = np.stack(
        [np.asarray(res.results[i]["out"], dtype=np.float32) for i in range(8)]
    )  # [8, L, D]

    const = w_o @ b_qkv[2 * D:] + b_o  # [D]
    out = (partial.reshape(B, 4, L, D).sum(axis=1) / (SW * SW)
           + x + const[None, None, :])
    return out.astype(np.float32)
